# revision 16
# baseline (speedup 1.0000x reference)
"""Trainium2 Bass kernel for nn_AttnInteractionLayer_2851858284689.

Math note: the reference's einsum ``'rfdh,rfoh->rfoh'`` contracts alpha over
its *softmax* axis (the labels are shifted relative to alpha's real layout
(r, d, f, h)), and softmax sums to one along that axis.  The attention output
therefore collapses exactly to ``vals``, and the whole module reduces to

    out = LayerNorm( leaky_relu( x @ (W_v.reshape(256,512) + W_r) ) ) * gamma + beta

which is what this kernel computes (verified < 1e-6 rel err vs the reference
in fp32; the bf16 device pipeline lands at ~5e-3).

Distribution: pure data parallel over the 4096*32 = 131072 (row, field)
tokens: 16384 tokens per NeuronCore, weights replicated.  The per-core shard
of x is pre-transposed and bf16-cast on the host so the contraction axis
lands on SBUF partitions.

Device kernel per core (tokens in 16 blocks of 1024, two 512-token groups):
  - PE:   bf16 matmuls  y[128t, 512j] += xT[k,128t].T @ W[k, 512j], fp32 PSUM
  - ACT:  merged leaky_relu (Prelu alpha=.01) over [128, 4*512] PSUM -> bf16
          SBUF per group, batched sqrt(var+eps), and half the normalizes as
          Identity(y*rstd - mean*rstd); Prelu/Sqrt/Identity share one
          act-table set, so exactly one table load
  - DVE:  one multi-group bn_stats per group, bn_aggr per sub-tile,
          reciprocal, the other half of the normalizes in packed-bf16
          tensor_scalar
  - DMA:  bf16 x in, bf16 y out (host upcasts to fp32); ~24 MB/core HBM
"""

import numpy as np
import ml_dtypes

import concourse.bass as bass
import concourse.tile as tile
from concourse import bacc, mybir
from concourse.bass_utils import run_bass_kernel_spmd


def _ensure_ntff_hook():
    """This image lacks ``antenv.axon_hooks``; inject it (ctypes on
    libaxon_pjrt.so) so run_bass_kernel_spmd(trace=True) / BASS_TRACE=1
    works instead of raising ImportError."""
    try:
        from antenv.axon_hooks import get_axon_ntff_profile_hook  # noqa: F401
        return
    except ImportError:
        pass
    try:
        import contextlib
        import ctypes
        import sys
        import types

        lib = ctypes.CDLL("/opt/axon/libaxon_pjrt.so")
        if not hasattr(lib, "axon_start_nrt_profile"):
            return
        lib.axon_start_nrt_profile.argtypes = [
            ctypes.POINTER(ctypes.c_int64), ctypes.c_size_t]
        lib.axon_start_nrt_profile.restype = ctypes.c_int64
        lib.axon_stop_nrt_profile.argtypes = [ctypes.c_char_p]
        lib.axon_stop_nrt_profile.restype = ctypes.c_int64

        @contextlib.contextmanager
        def _hook(output_dir, device_ids):
            import jax
            jax.devices()
            if device_ids:
                ids = (ctypes.c_int64 * len(device_ids))(*device_ids)
                rc = lib.axon_start_nrt_profile(ids, len(device_ids))
            else:
                rc = lib.axon_start_nrt_profile(None, 0)
            if rc != 0:
                raise RuntimeError(f"axon_start_nrt_profile rc={rc}")
            try:
                yield
            finally:
                lib.axon_stop_nrt_profile(str(output_dir).encode())

        import antenv
        mod = types.ModuleType("antenv.axon_hooks")
        mod.get_axon_ntff_profile_hook = lambda: _hook
        mod.set_axon_ntff_profile_hook = lambda h: None
        sys.modules["antenv.axon_hooks"] = mod
        antenv.axon_hooks = mod
    except Exception:
        pass


_ensure_ntff_hook()

R, F, IN, OUT_TOT = 4096, 32, 256, 512
N_CORES = 8
TOKENS = R * F
TPC = TOKENS // N_CORES          # tokens per core: 16384
KC = IN // 128                   # contraction chunks: 2
BLK = 1024                       # token block
NBLK = TPC // BLK                # 16
GRP = 4                          # sub-tiles per merged-Prelu group (4 PSUM banks)
NGRP = BLK // (GRP * 128)        # 2 groups per block
SUB = BLK // 128                 # 8 sub-tiles per block
EPS = 1e-5
NEG_SLOPE = 0.01
BF16 = mybir.dt.bfloat16
F32 = mybir.dt.float32

_compiled = {}


def _build_nc():
    nc = bacc.Bacc(None)
    xT = nc.declare_dram_parameter("xT", [KC, 128, TPC], BF16, isOutput=False)
    w = nc.declare_dram_parameter("w", [KC, 128, OUT_TOT], BF16, isOutput=False)
    y = nc.declare_dram_parameter("y", [TPC, OUT_TOT], BF16, isOutput=True)

    with tile.TileContext(nc) as tc:
        with (
            tc.tile_pool(name="singles", bufs=1) as singles,
            tc.tile_pool(name="xpool", bufs=4) as xpool,
            tc.tile_pool(name="ypool", bufs=4) as ypool,
            tc.tile_pool(name="opool", bufs=4) as opool,
            tc.tile_pool(name="stats", bufs=8) as stats_pool,
            tc.tile_pool(name="psum", bufs=2, space="PSUM") as psum,
        ):
            w_sb = singles.tile([128, KC, OUT_TOT], BF16)
            nc.sync.dma_start(out=w_sb, in_=w[:].rearrange("c k n -> k c n"))
            eps_sb = singles.tile([128, 1], F32)
            nc.vector.memset(eps_sb, EPS)

            for b in range(NBLK):
                x_sb = xpool.tile([128, KC, BLK], BF16)
                nc.sync.dma_start(
                    out=x_sb,
                    in_=xT[:, :, b * BLK:(b + 1) * BLK].rearrange("c k t -> k c t"),
                )
                y_sb = ypool.tile([128, SUB, OUT_TOT], BF16)
                o_sb = opool.tile([128, SUB, OUT_TOT], BF16)
                mv_all = stats_pool.tile([128, SUB, 2], F32)
                st_all = stats_pool.tile([128, SUB, 6], F32)

                # block 0 uses single-bank groups so the first Prelu starts
                # after 2 matmuls instead of 8 (shrinks the pipeline-fill gap)
                grp = 1 if b == 0 else GRP
                for g in range(SUB // grp):
                    ps_full = psum.tile([128, GRP, OUT_TOT], F32, tag="ps")
                    ps = ps_full[:, :grp, :]
                    for j in range(grp):
                        i = g * grp + j
                        nc.tensor.matmul(
                            ps[:, j, :], lhsT=x_sb[:, 0, bass.ts(i, 128)],
                            rhs=w_sb[:, 0, :], start=True, stop=False,
                        )
                        nc.tensor.matmul(
                            ps[:, j, :], lhsT=x_sb[:, 1, bass.ts(i, 128)],
                            rhs=w_sb[:, 1, :], start=False, stop=True,
                        )
                    nc.scalar.activation(
                        y_sb[:, g * grp:(g + 1) * grp, :], ps,
                        mybir.ActivationFunctionType.Prelu, alpha=NEG_SLOPE,
                    )
                    for j in range(grp):
                        i = g * grp + j
                        nc.vector.bn_stats(st_all[:, i, :], y_sb[:, i, :])

                for i in range(SUB):
                    nc.vector.bn_aggr(mv_all[:, i, :], st_all[:, i, :])

                std_all = stats_pool.tile([128, SUB], F32)
                nc.scalar.activation(
                    std_all, mv_all[:, :, 1], mybir.ActivationFunctionType.Sqrt,
                    bias=eps_sb,
                )
                rstd_all = stats_pool.tile([128, SUB], F32)
                nc.vector.reciprocal(rstd_all, std_all)
                # -mean*rstd for the ACT-normalized sub-tiles
                nmr_all = stats_pool.tile([128, SUB], F32)
                nc.vector.tensor_tensor(
                    nmr_all, mv_all[:, :, 0], rstd_all, mybir.AluOpType.mult,
                )
                nc.vector.tensor_scalar_mul(nmr_all, nmr_all, -1.0)

                for i in range(SUB):
                    if i % 2 == 0:
                        nc.vector.tensor_scalar(
                            o_sb[:, i, :], y_sb[:, i, :],
                            scalar1=mv_all[:, i, 0:1],
                            scalar2=rstd_all[:, i:i + 1],
                            op0=mybir.AluOpType.subtract,
                            op1=mybir.AluOpType.mult,
                        )
                    else:
                        nc.scalar.activation(
                            o_sb[:, i, :], y_sb[:, i, :],
                            mybir.ActivationFunctionType.Identity,
                            bias=nmr_all[:, i:i + 1],
                            scale=rstd_all[:, i:i + 1],
                        )
                nc.sync.dma_start(
                    out=y[b * BLK:(b + 1) * BLK, :].rearrange(
                        "(i p) j -> p i j", p=128),
                    in_=o_sb,
                )
    nc.finalize()
    return nc


def _get_nc():
    if "nc" not in _compiled:
        _compiled["nc"] = _build_nc()
    return _compiled["nc"]


def _in_maps(x, W_v, W_r):
    x = np.asarray(x, dtype=np.float32)
    W = (np.asarray(W_v, dtype=np.float32).reshape(IN, OUT_TOT)
         + np.asarray(W_r, dtype=np.float32))
    w_dev = np.ascontiguousarray(
        W.reshape(KC, 128, OUT_TOT).astype(ml_dtypes.bfloat16))

    xs = x.reshape(TOKENS, IN)
    in_maps = []
    for c in range(N_CORES):
        shard = xs[c * TPC:(c + 1) * TPC]                      # [TPC, IN]
        xT = np.ascontiguousarray(shard.T.astype(ml_dtypes.bfloat16))
        in_maps.append({"xT": xT.reshape(KC, 128, TPC), "w": w_dev})
    return in_maps


def _gather(res):
    out = np.concatenate([res.results[c]["y"] for c in range(N_CORES)], axis=0)
    return out.reshape(R, F, OUT_TOT).astype(np.float32)


def kernel(x, W_q, W_k, W_v, W_r, ln_gamma, ln_beta):
    nc = _get_nc()
    in_maps = _in_maps(x, W_v, W_r)
    res = run_bass_kernel_spmd(nc, in_maps, list(range(N_CORES)))
    out = _gather(res)

    gamma = np.asarray(ln_gamma, dtype=np.float32)
    beta = np.asarray(ln_beta, dtype=np.float32)
    if not (np.all(gamma == 1.0) and np.all(beta == 0.0)):
        # LN affine is the final op of the reference; fold it on the host in
        # the (never-hit-in-practice) case of non-trivial gamma/beta.
        out = out * gamma + beta
    return out.astype(np.float32)


# revision 18
# speedup vs baseline: 1.0160x; 1.0160x over previous
"""Trainium2 Bass kernel for nn_AttnInteractionLayer_2851858284689.

Math note: the reference's einsum ``'rfdh,rfoh->rfoh'`` contracts alpha over
its *softmax* axis (the labels are shifted relative to alpha's real layout
(r, d, f, h)), and softmax sums to one along that axis.  The attention output
therefore collapses exactly to ``vals``, and the whole module reduces to

    out = LayerNorm( leaky_relu( x @ (W_v.reshape(256,512) + W_r) ) ) * gamma + beta

which is what this kernel computes (verified < 1e-6 rel err vs the reference
in fp32; the bf16 device pipeline lands at ~5e-3).

Distribution: pure data parallel over the 4096*32 = 131072 (row, field)
tokens: 16384 tokens per NeuronCore, weights replicated.  The per-core shard
of x is pre-transposed and bf16-cast on the host so the contraction axis
lands on SBUF partitions.

Device kernel per core (tokens in 16 blocks of 1024, two 512-token groups):
  - PE:   bf16 matmuls  y[128t, 512j] += xT[k,128t].T @ W[k, 512j], fp32 PSUM
  - ACT:  merged leaky_relu (Prelu alpha=.01) over [128, 4*512] PSUM -> bf16
          SBUF per group, batched sqrt(var+eps), and half the normalizes as
          Identity(y*rstd - mean*rstd); Prelu/Sqrt/Identity share one
          act-table set, so exactly one table load
  - DVE:  one multi-group bn_stats per group, bn_aggr per sub-tile,
          reciprocal, the other half of the normalizes in packed-bf16
          tensor_scalar
  - DMA:  bf16 x in, bf16 y out (host upcasts to fp32); ~24 MB/core HBM
"""

import numpy as np
import ml_dtypes

import concourse.bass as bass
import concourse.tile as tile
from concourse import bacc, mybir
from concourse.bass_utils import run_bass_kernel_spmd


def _ensure_ntff_hook():
    """This image lacks ``antenv.axon_hooks``; inject it (ctypes on
    libaxon_pjrt.so) so run_bass_kernel_spmd(trace=True) / BASS_TRACE=1
    works instead of raising ImportError."""
    try:
        from antenv.axon_hooks import get_axon_ntff_profile_hook  # noqa: F401
        return
    except ImportError:
        pass
    try:
        import contextlib
        import ctypes
        import sys
        import types

        lib = ctypes.CDLL("/opt/axon/libaxon_pjrt.so")
        if not hasattr(lib, "axon_start_nrt_profile"):
            return
        lib.axon_start_nrt_profile.argtypes = [
            ctypes.POINTER(ctypes.c_int64), ctypes.c_size_t]
        lib.axon_start_nrt_profile.restype = ctypes.c_int64
        lib.axon_stop_nrt_profile.argtypes = [ctypes.c_char_p]
        lib.axon_stop_nrt_profile.restype = ctypes.c_int64

        @contextlib.contextmanager
        def _hook(output_dir, device_ids):
            import jax
            jax.devices()
            if device_ids:
                ids = (ctypes.c_int64 * len(device_ids))(*device_ids)
                rc = lib.axon_start_nrt_profile(ids, len(device_ids))
            else:
                rc = lib.axon_start_nrt_profile(None, 0)
            if rc != 0:
                raise RuntimeError(f"axon_start_nrt_profile rc={rc}")
            try:
                yield
            finally:
                lib.axon_stop_nrt_profile(str(output_dir).encode())

        import antenv
        mod = types.ModuleType("antenv.axon_hooks")
        mod.get_axon_ntff_profile_hook = lambda: _hook
        mod.set_axon_ntff_profile_hook = lambda h: None
        sys.modules["antenv.axon_hooks"] = mod
        antenv.axon_hooks = mod
    except Exception:
        pass


_ensure_ntff_hook()

R, F, IN, OUT_TOT = 4096, 32, 256, 512
N_CORES = 8
TOKENS = R * F
TPC = TOKENS // N_CORES          # tokens per core: 16384
KC = IN // 128                   # contraction chunks: 2
BLK = 1024                       # token block
NBLK = TPC // BLK                # 16
GRP = 4                          # sub-tiles per merged-Prelu group (4 PSUM banks)
NGRP = BLK // (GRP * 128)        # 2 groups per block
SUB = BLK // 128                 # 8 sub-tiles per block
EPS = 1e-5
NEG_SLOPE = 0.01
BF16 = mybir.dt.bfloat16
F32 = mybir.dt.float32

_compiled = {}


def _build_nc():
    nc = bacc.Bacc(None)
    xT = nc.declare_dram_parameter("xT", [KC, 128, TPC], BF16, isOutput=False)
    w = nc.declare_dram_parameter("w", [KC, 128, OUT_TOT], BF16, isOutput=False)
    y = nc.declare_dram_parameter("y", [TPC, OUT_TOT], BF16, isOutput=True)

    with tile.TileContext(nc) as tc:
        with (
            tc.tile_pool(name="singles", bufs=1) as singles,
            tc.tile_pool(name="xpool", bufs=6) as xpool,
            tc.tile_pool(name="ypool", bufs=6) as ypool,
            tc.tile_pool(name="opool", bufs=6) as opool,
            tc.tile_pool(name="stats", bufs=10) as stats_pool,
            tc.tile_pool(name="psum", bufs=2, space="PSUM") as psum,
        ):
            w_sb = singles.tile([128, KC, OUT_TOT], BF16)
            nc.sync.dma_start(out=w_sb, in_=w[:].rearrange("c k n -> k c n"))
            eps_sb = singles.tile([128, 1], F32)
            nc.vector.memset(eps_sb, EPS)

            for b in range(NBLK):
                x_sb = xpool.tile([128, KC, BLK], BF16)
                nc.sync.dma_start(
                    out=x_sb,
                    in_=xT[:, :, b * BLK:(b + 1) * BLK].rearrange("c k t -> k c t"),
                )
                y_sb = ypool.tile([128, SUB, OUT_TOT], BF16)
                o_sb = opool.tile([128, SUB, OUT_TOT], BF16)
                mv_all = stats_pool.tile([128, SUB, 2], F32)
                st_all = stats_pool.tile([128, SUB, 6], F32)

                for g in range(NGRP):
                    ps = psum.tile([128, GRP, OUT_TOT], F32)  # 4 banks
                    for j in range(GRP):
                        i = g * GRP + j
                        nc.tensor.matmul(
                            ps[:, j, :], lhsT=x_sb[:, 0, bass.ts(i, 128)],
                            rhs=w_sb[:, 0, :], start=True, stop=False,
                        )
                        nc.tensor.matmul(
                            ps[:, j, :], lhsT=x_sb[:, 1, bass.ts(i, 128)],
                            rhs=w_sb[:, 1, :], start=False, stop=True,
                        )
                    nc.scalar.activation(
                        y_sb[:, g * GRP:(g + 1) * GRP, :], ps,
                        mybir.ActivationFunctionType.Prelu, alpha=NEG_SLOPE,
                    )
                    for j in range(GRP):
                        i = g * GRP + j
                        nc.vector.bn_stats(st_all[:, i, :], y_sb[:, i, :])

                for i in range(SUB):
                    nc.vector.bn_aggr(mv_all[:, i, :], st_all[:, i, :])

                std_all = stats_pool.tile([128, SUB], F32)
                nc.scalar.activation(
                    std_all, mv_all[:, :, 1], mybir.ActivationFunctionType.Sqrt,
                    bias=eps_sb,
                )
                rstd_all = stats_pool.tile([128, SUB], F32)
                nc.vector.reciprocal(rstd_all, std_all)
                # -mean*rstd for the ACT-normalized sub-tiles
                nmr_all = stats_pool.tile([128, SUB], F32)
                nc.vector.tensor_tensor(
                    nmr_all, mv_all[:, :, 0], rstd_all, mybir.AluOpType.mult,
                )
                nc.vector.tensor_scalar_mul(nmr_all, nmr_all, -1.0)

                for i in range(SUB):
                    if i % 2 == 0:
                        nc.vector.tensor_scalar(
                            o_sb[:, i, :], y_sb[:, i, :],
                            scalar1=mv_all[:, i, 0:1],
                            scalar2=rstd_all[:, i:i + 1],
                            op0=mybir.AluOpType.subtract,
                            op1=mybir.AluOpType.mult,
                        )
                    else:
                        nc.scalar.activation(
                            o_sb[:, i, :], y_sb[:, i, :],
                            mybir.ActivationFunctionType.Identity,
                            bias=nmr_all[:, i:i + 1],
                            scale=rstd_all[:, i:i + 1],
                        )
                nc.sync.dma_start(
                    out=y[b * BLK:(b + 1) * BLK, :].rearrange(
                        "(i p) j -> p i j", p=128),
                    in_=o_sb,
                )
    nc.finalize()
    return nc


def _get_nc():
    if "nc" not in _compiled:
        _compiled["nc"] = _build_nc()
    return _compiled["nc"]


def _in_maps(x, W_v, W_r):
    x = np.asarray(x, dtype=np.float32)
    W = (np.asarray(W_v, dtype=np.float32).reshape(IN, OUT_TOT)
         + np.asarray(W_r, dtype=np.float32))
    w_dev = np.ascontiguousarray(
        W.reshape(KC, 128, OUT_TOT).astype(ml_dtypes.bfloat16))

    xs = x.reshape(TOKENS, IN)
    in_maps = []
    for c in range(N_CORES):
        shard = xs[c * TPC:(c + 1) * TPC]                      # [TPC, IN]
        xT = np.ascontiguousarray(shard.T.astype(ml_dtypes.bfloat16))
        in_maps.append({"xT": xT.reshape(KC, 128, TPC), "w": w_dev})
    return in_maps


def _gather(res):
    out = np.concatenate([res.results[c]["y"] for c in range(N_CORES)], axis=0)
    return out.reshape(R, F, OUT_TOT).astype(np.float32)


def kernel(x, W_q, W_k, W_v, W_r, ln_gamma, ln_beta):
    nc = _get_nc()
    in_maps = _in_maps(x, W_v, W_r)
    res = run_bass_kernel_spmd(nc, in_maps, list(range(N_CORES)))
    out = _gather(res)

    gamma = np.asarray(ln_gamma, dtype=np.float32)
    beta = np.asarray(ln_beta, dtype=np.float32)
    if not (np.all(gamma == 1.0) and np.all(beta == 0.0)):
        # LN affine is the final op of the reference; fold it on the host in
        # the (never-hit-in-practice) case of non-trivial gamma/beta.
        out = out * gamma + beta
    return out.astype(np.float32)
